# revision 17
# baseline (speedup 1.0000x reference)
"""Trainium2 Bass kernel for nn_Conv2DMod (StyleGAN2-style modulated 3x3 conv).

Problem: x[8,64,256,256], s[8,64], weight[64,64,3,3] (f32)
  w = weight * (s+1) per sample; demod by rsqrt(sum w^2 over (Cin,K,K));
  out[b] = conv2d(x[b], w_b, pad=1).

Sharding: data-parallel over batch. 8 samples -> 8 NeuronCores, one each.

Per-core algorithm (V5):
  - host pre-pads x to [64,258,258] bf16 (zero halo rows+cols), pre-transposes
    weight to lhsT layout [i, p, o] replicated to 128 partitions, s as column.
  - all 4 x row-slab loads are issued first on the sync (HWDGE) queue
    (xpool bufs=4, so none waits on buffer recycling); weight/s prep inputs go
    on the scalar queue so the two never serialize.
  - conv runs with MODULATED but UNdemodulated weights (w2 = wT*(s+1), bf16),
    ready as soon as the weight DMA + one DVE pass finish. The demod scale
    d[o] = rsqrt(sum w^2) is computed concurrently as a [128,1] column
    (duplicated halves, via two K=1 transpose matmuls) and applied during
    evacuation as a per-partition scale.
  - conv as shift-matmul over 9 kernel positions, 4 independent 64x64 PE cells
    (row tiles = block0/block1 x data, col tiles = even/odd kernel positions,
    crossed psum banks). Groups of 2 chunks (2x2 output rows, N=512 each)
    share each LDWEIGHTS; position 8 alternates col groups per chunk so every
    cell runs exactly 9 matmuls per group.
  - psum group tiles span 2 banks; 2 bufs x (E,O) = all 8 banks.
  - evacuation per group, engines balanced under the PE's ~2.2us/group:
      DVE: stageB = psE * d2   (tensor_scalar 2x rate from PSUM, frees psE)
      ACT: stage[0:64]  = psO[64:128] * d2 (cross-base copy w/ scale)
      ACT: stage[64:128] = psO[0:64]  * d2 (frees psO)
      DVE: stage += stageB     (aligned full-width bf16 add, 2x rate)
    Output DMA'd as bf16 (host upcasts); stores on SWDGE (gpsimd queue).
"""

import ml_dtypes
import numpy as np

import concourse.bacc as bacc
import concourse.mybir as mybir
import concourse.tile as tile
from concourse.bass import ts
from concourse.bass_utils import run_bass_kernel_spmd

F32 = mybir.dt.float32
BF16 = mybir.dt.bfloat16

B, CIN, COUT, KK, H, W = 8, 64, 64, 3, 256, 256
EPS = 1e-8
PW = W + 2          # padded row width
PH = H + 2          # padded height
HB = 32             # output rows per block
NBI = H // (2 * HB)  # pair-iterations (4)
NGRP = HB // 4      # 2-chunk groups per iteration (8)
FLUSH_G = 2         # groups per stage flush (4 chunks = 8 rows per block)

EVEN = [0, 2, 4, 6]
ODD = [1, 3, 5, 7]


def emit_x_load(nc, xt, xp, i):
    # block0 on the sync (HWDGE) queue, block1 on the gpsimd (SWDGE) queue:
    # queues serialize doorbell->completion per DMA, so the two halves must
    # stream on independent queues to land in parallel
    lo0 = 64 * i          # block0 padded rows [lo0, lo0+34)
    lo1 = 64 * i + HB     # block1 padded rows [lo1, lo1+34)
    if i == 0:
        # split so the first groups' rows land fast
        nc.sync.dma_start(out=xt[0:64, 0:12, :], in_=xp[:, 0:12, :])
        nc.gpsimd.dma_start(out=xt[64:128, 0:12, :], in_=xp[:, lo1:lo1 + 12, :])
        nc.sync.dma_start(out=xt[0:64, 12:HB + 2, :], in_=xp[:, 12:HB + 2, :])
        nc.gpsimd.dma_start(out=xt[64:128, 12:HB + 2, :],
                            in_=xp[:, lo1 + 12:lo1 + HB + 2, :])
    else:
        nc.sync.dma_start(out=xt[0:64, :, :], in_=xp[:, lo0:lo0 + HB + 2, :])
        nc.gpsimd.dma_start(out=xt[64:128, :, :],
                            in_=xp[:, lo1:lo1 + HB + 2, :])


def build_nc():
    nc = bacc.Bacc("TRN2")
    xp = nc.dram_tensor("xp", [CIN, PH, PW], BF16, kind="ExternalInput")
    sT = nc.dram_tensor("sT", [128, 1], F32, kind="ExternalInput")
    wgtT = nc.dram_tensor("wgtT", [128, 9 * 64], F32, kind="ExternalInput")
    out = nc.dram_tensor("out", [COUT, H, W], BF16, kind="ExternalOutput")

    with (
        tile.TileContext(nc) as tc,
        tc.tile_pool(name="const", bufs=1) as constp,
        tc.tile_pool(name="xpool", bufs=NBI) as xpool,
        tc.tile_pool(name="prep", bufs=1) as prepp,
    ):
        w2 = constp.tile([128, 9, 64], BF16)   # [i, p, o] modulated lhsT
        d2 = constp.tile([128, 1], F32)        # demod scale column (dup halves)

        # weight/s inputs first (small), then x slab loads, all on the sync
        # HWDGE queue which streams them in program order from t=0
        wT = prepp.tile([128, 9, 64], F32)    # [i, p, o]
        nc.sync.dma_start(out=wT[:, :, :], in_=wgtT[:, :])
        sp1 = prepp.tile([128, 1], F32)
        nc.sync.dma_start(out=sp1[:, :], in_=sT[:, :])
        xts = []
        for i in range(NBI):
            xts.append(xpool.tile([128, HB + 2, PW], BF16, name=f"xt{i}",
                                  tag="xt"))
            emit_x_load(nc, xts[i], xp, i)

        # ---- weight prep (f32 math, bf16 result) ----
        # The bf16 w2 copy is emitted LAST so the conv LDWEIGHTS only become
        # ready after the prep matmuls are already in the tensor queue --
        # otherwise Tile schedules the conv stream first and the first
        # evacuations deadlock-stall on d2 for tens of us.
        with tc.tile_pool(name="prep_ps", bufs=1, space="PSUM") as prep_ps:
            nc.vector.tensor_scalar_add(sp1[:, :], sp1[:, :], 1.0)
            wmodF = prepp.tile([128, 9, 64], F32)
            nc.vector.tensor_scalar_mul(wmodF[:, :, :], wT[:, :, :],
                                        sp1[:, :])
            # demod norm: d[o] = rsqrt(sum_i sum_p wmod^2 + eps)
            sq = prepp.tile([64, 9, 64], F32)
            nc.vector.tensor_mul(sq[:, :, :], wmodF[0:64, :, :],
                                 wmodF[0:64, :, :])
            ones = prepp.tile([64, 1], F32)
            nc.vector.memset(ones[:, :], 1.0)
            psA = prep_ps.tile([64, 512], F32)
            psB = prep_ps.tile([64, 64], F32)
            nc.tensor.matmul(psA[0:1, :], ones[:, 0:1], sq[:, 0:8, :],
                             start=True, stop=True)
            nc.tensor.matmul(psB[0:1, :], ones[:, 0:1], sq[:, 8, :],
                             start=True, stop=True)
            acc = prepp.tile([1, 64], F32)
            nc.vector.tensor_copy(acc[0:1, :], psA[0:1, 0:64])
            for k in range(1, 8):
                nc.vector.tensor_add(acc[0:1, :], acc[0:1, :],
                                     psA[0:1, ts(k, 64)])
            nc.vector.tensor_add(acc[0:1, :], acc[0:1, :], psB[0:1, :])
            epst = prepp.tile([1, 1], F32)
            nc.vector.memset(epst[:, :], EPS)
            dtmp = prepp.tile([1, 64], F32)
            nc.scalar.activation(dtmp[0:1, :], acc[0:1, :],
                                 mybir.ActivationFunctionType.Sqrt,
                                 bias=epst[0:1, 0:1])
            dinv = prepp.tile([1, 64], F32)
            nc.vector.reciprocal(dinv[0:1, :], dtmp[0:1, :])
            # row -> column, duplicated to both halves, via tiny transpose
            # DMAs on the (otherwise idle) scalar-hosted queue
            nc.scalar.dma_start(out=d2[0:64, 0:1], in_=dinv[0:1, :])
            nc.scalar.dma_start(out=d2[64:128, 0:1], in_=dinv[0:1, :])
            # conv weights (bf16), demod applied at evacuation instead
            nc.vector.tensor_copy(w2[:, :, :], wmodF[:, :, :])

        # ---- main conv loop ----
        with (
            tc.tile_pool(name="stpool", bufs=3) as stpool,
            tc.tile_pool(name="pspool", bufs=2, space="PSUM") as pspool,
        ):
            for i in range(NBI):
                xt = xts[i]
                for gg in range(NGRP // FLUSH_G):
                    stage = stpool.tile([128, FLUSH_G * 1024], BF16,
                                        name=f"stage{i}_{gg}", tag="stage")
                    for gj in range(FLUSH_G):
                        g = gg * FLUSH_G + gj
                        psE = pspool.tile([128, 1024], F32,
                                          name=f"psE{i}_{g}", tag="psE")
                        psO = pspool.tile([128, 1024], F32,
                                          name=f"psO{i}_{g}", tag="psO")
                        # cells: (b, col h0)=even pos, (b, col h64)=odd pos
                        #  b0 even->psE[0:64], b1 even->psO[0:64]
                        #  b0 odd ->psO[64:128], b1 odd->psE[64:128]
                        for k in range(5):
                            for par in range(2):   # 0=col h0, 1=col h64
                                if k == 4:
                                    p = 8
                                    jjs = [par]    # p=8: jj0 on h0, jj1 on h64
                                else:
                                    p = (EVEN, ODD)[par][k]
                                    jjs = [0, 1]
                                dy, dx = divmod(p, 3)
                                for b in range(2):
                                    if par == 0:
                                        ps = (psE, psO)[b]
                                        pr = slice(0, 64)
                                        tp = (64 * b, 0)
                                    else:
                                        ps = (psO, psE)[b]
                                        pr = slice(64, 128)
                                        tp = (64 * b, 64)
                                    wap = w2[64 * b:64 * b + 64, p, :]
                                    nc.tensor.ldweights(wap, tile_position=tp)
                                    for jj in jjs:
                                        c = 2 * g + jj
                                        # evens: jj0 k0..4 (5), jj1 k0..3 (4)
                                        # odds:  jj0 k0..3 (4), jj1 k0..4 (5)
                                        st = (k == 0)
                                        if par == 0:
                                            sp = (k == 4) if jj == 0 else (k == 3)
                                        else:
                                            sp = (k == 3) if jj == 0 else (k == 4)
                                        nc.tensor.matmul(
                                            ps[pr, ts(jj, 512)], wap,
                                            xt[64 * b:64 * b + 64,
                                               2 * c + dy:2 * c + dy + 2,
                                               dx:dx + W],
                                            start=st, stop=sp,
                                            tile_position=tp,
                                        )
                        # evacuate group: stage rows [4*gj .. 4*gj+4)
                        dst = stage[:, ts(gj, 1024)]
                        nc.scalar.activation(dst[0:64, :], psO[64:128, :],
                                             mybir.ActivationFunctionType.Copy,
                                             scale=d2[64:128, :])
                        nc.scalar.activation(dst[64:128, :], psO[0:64, :],
                                             mybir.ActivationFunctionType.Copy,
                                             scale=d2[0:64, :])
                        nc.vector.scalar_tensor_tensor(
                            dst, psE[:, :], d2[:, :], dst,
                            op0=mybir.AluOpType.mult,
                            op1=mybir.AluOpType.add)
                    # flush: one DMA per block, 8 rows x 256 each
                    for b in range(2):
                        r0 = 64 * i + HB * b + 4 * FLUSH_G * gg
                        nc.gpsimd.dma_start(
                            out=out[:, r0:r0 + 4 * FLUSH_G, :],
                            in_=stage[64 * b:64 * b + 64, :],
                        )
    nc.finalize()
    return nc


_NC = None


def _get_nc():
    global _NC
    if _NC is None:
        _NC = build_nc()
    return _NC


def make_in_maps(x, s, weight):
    x = np.asarray(x, dtype=np.float32)
    s = np.asarray(s, dtype=np.float32)
    w = np.asarray(weight, dtype=np.float32)
    wT = w.reshape(COUT, CIN, 9).transpose(1, 2, 0).reshape(CIN, 9 * COUT)
    wT2 = np.ascontiguousarray(np.concatenate([wT, wT], axis=0))  # [128, 576]
    xpad = np.zeros((B, CIN, PH, PW), dtype=ml_dtypes.bfloat16)
    xpad[:, :, 1:H + 1, 1:W + 1] = x
    maps = []
    for c in range(B):
        sT2 = np.ascontiguousarray(
            np.tile(s[c][:, None], (2, 1)).astype(np.float32))  # [128, 1]
        maps.append({"xp": xpad[c], "sT": sT2, "wgtT": wT2})
    return maps


def run(x, s, weight, **kw):
    nc = _get_nc()
    res = run_bass_kernel_spmd(nc, make_in_maps(x, s, weight),
                               core_ids=list(range(B)), **kw)
    out = np.stack([np.asarray(r["out"]) for r in res.results])
    return out, res


def kernel(x, s, weight):
    out, _ = run(x, s, weight)
    return out.astype(np.float32)


if __name__ == "__main__":
    rng = np.random.default_rng(0)
    xv = rng.standard_normal((B, CIN, H, W), dtype=np.float32)
    sv = rng.standard_normal((B, CIN), dtype=np.float32)
    wv = (rng.standard_normal((COUT, CIN, KK, KK), dtype=np.float32)
          * np.float32(np.sqrt(2.0 / (CIN * KK * KK))))
    o = kernel(xv, sv, wv)
    print("ran ok", o.shape, o.dtype, float(np.abs(o).max()))


# revision 18
# speedup vs baseline: 1.0853x; 1.0853x over previous
"""Trainium2 Bass kernel for nn_Conv2DMod (StyleGAN2-style modulated 3x3 conv).

Problem: x[8,64,256,256], s[8,64], weight[64,64,3,3] (f32)
  w = weight * (s+1) per sample; demod by rsqrt(sum w^2 over (Cin,K,K));
  out[b] = conv2d(x[b], w_b, pad=1).

Sharding: data-parallel over batch. 8 samples -> 8 NeuronCores, one each.

Per-core algorithm (V5):
  - host pre-pads x to [64,258,258] bf16 (zero halo rows+cols), pre-transposes
    weight to lhsT layout [i, p, o] replicated to 128 partitions, s as column.
  - all 4 x row-slab loads are issued first on the sync (HWDGE) queue
    (xpool bufs=4, so none waits on buffer recycling); weight/s prep inputs go
    on the scalar queue so the two never serialize.
  - conv runs with MODULATED but UNdemodulated weights (w2 = wT*(s+1), bf16),
    ready as soon as the weight DMA + one DVE pass finish. The demod scale
    d[o] = rsqrt(sum w^2) is computed concurrently as a [128,1] column
    (duplicated halves, via two K=1 transpose matmuls) and applied during
    evacuation as a per-partition scale.
  - conv as shift-matmul over 9 kernel positions, 4 independent 64x64 PE cells
    (row tiles = block0/block1 x data, col tiles = even/odd kernel positions,
    crossed psum banks). Groups of 2 chunks (2x2 output rows, N=512 each)
    share each LDWEIGHTS; position 8 alternates col groups per chunk so every
    cell runs exactly 9 matmuls per group.
  - psum group tiles span 2 banks; 2 bufs x (E,O) = all 8 banks.
  - evacuation per group, engines balanced under the PE's ~2.2us/group:
      DVE: stageB = psE * d2   (frees psE early, independent of ACT)
      ACT: stage[0:64]  = psO[64:128] * d2 (cross-base copy w/ scale)
      ACT: stage[64:128] = psO[0:64]  * d2 (frees psO)
      DVE: stage += stageB     (aligned full-width bf16 add, 2x rate)
    Output DMA'd as bf16 (host upcasts); stores on SWDGE (gpsimd queue).
"""

import ml_dtypes
import numpy as np

import concourse.bacc as bacc
import concourse.mybir as mybir
import concourse.tile as tile
from concourse.bass import ts
from concourse.bass_utils import run_bass_kernel_spmd

F32 = mybir.dt.float32
BF16 = mybir.dt.bfloat16

B, CIN, COUT, KK, H, W = 8, 64, 64, 3, 256, 256
EPS = 1e-8
PW = W + 2          # padded row width
PH = H + 2          # padded height
HB = 32             # output rows per block
NBI = H // (2 * HB)  # pair-iterations (4)
NGRP = HB // 4      # 2-chunk groups per iteration (8)
FLUSH_G = 2         # groups per stage flush (4 chunks = 8 rows per block)

EVEN = [0, 2, 4, 6]
ODD = [1, 3, 5, 7]


def emit_x_load(nc, xt, xp, i):
    lo0 = 64 * i          # block0 padded rows [lo0, lo0+34)
    lo1 = 64 * i + HB     # block1 padded rows [lo1, lo1+34)
    if i == 0:
        # split so the first groups' rows land fast
        nc.sync.dma_start(out=xt[0:64, 0:12, :], in_=xp[:, 0:12, :])
        nc.sync.dma_start(out=xt[64:128, 0:12, :], in_=xp[:, lo1:lo1 + 12, :])
        nc.sync.dma_start(out=xt[0:64, 12:HB + 2, :], in_=xp[:, 12:HB + 2, :])
        nc.sync.dma_start(out=xt[64:128, 12:HB + 2, :],
                          in_=xp[:, lo1 + 12:lo1 + HB + 2, :])
    else:
        nc.sync.dma_start(out=xt[0:64, :, :], in_=xp[:, lo0:lo0 + HB + 2, :])
        nc.sync.dma_start(out=xt[64:128, :, :], in_=xp[:, lo1:lo1 + HB + 2, :])


def build_nc():
    nc = bacc.Bacc("TRN2")
    xp = nc.dram_tensor("xp", [CIN, PH, PW], BF16, kind="ExternalInput")
    sT = nc.dram_tensor("sT", [128, 1], F32, kind="ExternalInput")
    wgtT = nc.dram_tensor("wgtT", [128, 9 * 64], F32, kind="ExternalInput")
    out = nc.dram_tensor("out", [COUT, H, W], BF16, kind="ExternalOutput")

    with (
        tile.TileContext(nc) as tc,
        tc.tile_pool(name="const", bufs=1) as constp,
        tc.tile_pool(name="xpool", bufs=NBI) as xpool,
    ):
        w2 = constp.tile([128, 9, 64], BF16)   # [i, p, o] modulated lhsT
        d2 = constp.tile([128, 1], F32)        # demod scale column (dup halves)

        # all x slab loads first in program order -> sync HWDGE queue streams
        # them back to back from t=0
        xts = []
        for i in range(NBI):
            xts.append(xpool.tile([128, HB + 2, PW], BF16, name=f"xt{i}",
                                  tag="xt"))
            emit_x_load(nc, xts[i], xp, i)

        # ---- weight prep (f32 math, bf16 result), no transposes ----
        with (
            tc.tile_pool(name="prep", bufs=1) as prepp,
            tc.tile_pool(name="prep_ps", bufs=1, space="PSUM") as prep_ps,
        ):
            wT = prepp.tile([128, 9, 64], F32)    # [i, p, o]
            nc.scalar.dma_start(out=wT[:, :, :], in_=wgtT[:, :])
            sp1 = prepp.tile([128, 1], F32)
            nc.scalar.dma_start(out=sp1[:, :], in_=sT[:, :])
            nc.vector.tensor_scalar_add(sp1[:, :], sp1[:, :], 1.0)
            # modulate straight into bf16 conv weights (demod applied at evac)
            nc.vector.tensor_scalar_mul(w2[:, :, :], wT[:, :, :], sp1[:, :])
            # demod norm: d2[o] = rsqrt(sum_i sum_p wmod^2 + eps) as a column
            wmod = prepp.tile([64, 9, 64], F32)
            nc.vector.tensor_scalar_mul(wmod[:, :, :], wT[0:64, :, :],
                                        sp1[0:64, :])
            sq = prepp.tile([64, 9, 64], F32)
            nc.vector.tensor_mul(sq[:, :, :], wmod[:, :, :], wmod[:, :, :])
            ones = prepp.tile([64, 1], F32)
            nc.vector.memset(ones[:, :], 1.0)
            psA = prep_ps.tile([64, 512], F32)
            psB = prep_ps.tile([64, 64], F32)
            nc.tensor.matmul(psA[0:1, :], ones[:, 0:1], sq[:, 0:8, :],
                             start=True, stop=True)
            nc.tensor.matmul(psB[0:1, :], ones[:, 0:1], sq[:, 8, :],
                             start=True, stop=True)
            acc = prepp.tile([1, 64], F32)
            nc.vector.tensor_copy(acc[0:1, :], psA[0:1, 0:64])
            for k in range(1, 8):
                nc.vector.tensor_add(acc[0:1, :], acc[0:1, :],
                                     psA[0:1, ts(k, 64)])
            nc.vector.tensor_add(acc[0:1, :], acc[0:1, :], psB[0:1, :])
            epst = prepp.tile([1, 1], F32)
            nc.vector.memset(epst[:, :], EPS)
            dtmp = prepp.tile([1, 64], F32)
            nc.scalar.activation(dtmp[0:1, :], acc[0:1, :],
                                 mybir.ActivationFunctionType.Sqrt,
                                 bias=epst[0:1, 0:1])
            dinv = prepp.tile([1, 64], F32)
            nc.vector.reciprocal(dinv[0:1, :], dtmp[0:1, :])
            # row -> column (both halves) via K=1 transpose matmuls
            ones1 = prepp.tile([1, 1], F32)
            nc.vector.memset(ones1[:, :], 1.0)
            psD = prep_ps.tile([128, 1], F32)
            nc.tensor.matmul(psD[0:64, 0:1], dinv[0:1, :], ones1[0:1, :],
                             start=True, stop=True)
            nc.tensor.matmul(psD[64:128, 0:1], dinv[0:1, :], ones1[0:1, :],
                             start=True, stop=True, tile_position=(0, 64))
            nc.vector.tensor_copy(d2[:, :], psD[:, :])

        # ---- main conv loop ----
        with (
            tc.tile_pool(name="stpool", bufs=2) as stpool,
            tc.tile_pool(name="sbpool", bufs=2) as sbpool,
            tc.tile_pool(name="pspool", bufs=2, space="PSUM") as pspool,
        ):
            for i in range(NBI):
                xt = xts[i]
                for gg in range(NGRP // FLUSH_G):
                    stage = stpool.tile([128, FLUSH_G * 1024], BF16,
                                        name=f"stage{i}_{gg}", tag="stage")
                    for gj in range(FLUSH_G):
                        g = gg * FLUSH_G + gj
                        psE = pspool.tile([128, 1024], F32,
                                          name=f"psE{i}_{g}", tag="psE")
                        psO = pspool.tile([128, 1024], F32,
                                          name=f"psO{i}_{g}", tag="psO")
                        # cells: (b, col h0)=even pos, (b, col h64)=odd pos
                        #  b0 even->psE[0:64], b1 even->psO[0:64]
                        #  b0 odd ->psO[64:128], b1 odd->psE[64:128]
                        for k in range(5):
                            for par in range(2):   # 0=col h0, 1=col h64
                                if k == 4:
                                    p = 8
                                    jjs = [par]    # p=8: jj0 on h0, jj1 on h64
                                else:
                                    p = (EVEN, ODD)[par][k]
                                    jjs = [0, 1]
                                dy, dx = divmod(p, 3)
                                for b in range(2):
                                    if par == 0:
                                        ps = (psE, psO)[b]
                                        pr = slice(0, 64)
                                        tp = (64 * b, 0)
                                    else:
                                        ps = (psO, psE)[b]
                                        pr = slice(64, 128)
                                        tp = (64 * b, 64)
                                    wap = w2[64 * b:64 * b + 64, p, :]
                                    nc.tensor.ldweights(wap, tile_position=tp)
                                    for jj in jjs:
                                        c = 2 * g + jj
                                        # evens: jj0 k0..4 (5), jj1 k0..3 (4)
                                        # odds:  jj0 k0..3 (4), jj1 k0..4 (5)
                                        st = (k == 0)
                                        if par == 0:
                                            sp = (k == 4) if jj == 0 else (k == 3)
                                        else:
                                            sp = (k == 3) if jj == 0 else (k == 4)
                                        nc.tensor.matmul(
                                            ps[pr, ts(jj, 512)], wap,
                                            xt[64 * b:64 * b + 64,
                                               2 * c + dy:2 * c + dy + 2,
                                               dx:dx + W],
                                            start=st, stop=sp,
                                            tile_position=tp,
                                        )
                        # evacuate group: stage rows [4*gj .. 4*gj+4)
                        dst = stage[:, ts(gj, 1024)]
                        stageB = sbpool.tile([128, 1024], BF16,
                                             name=f"stgB{i}_{g}", tag="stgB")
                        nc.vector.tensor_scalar_mul(stageB[:, :], psE[:, :],
                                                    d2[:, :])
                        nc.scalar.activation(dst[0:64, :], psO[64:128, :],
                                             mybir.ActivationFunctionType.Copy,
                                             scale=d2[64:128, :])
                        nc.scalar.activation(dst[64:128, :], psO[0:64, :],
                                             mybir.ActivationFunctionType.Copy,
                                             scale=d2[0:64, :])
                        nc.vector.tensor_add(dst, dst, stageB[:, :])
                    # flush: one DMA per block, 8 rows x 256 each
                    for b in range(2):
                        r0 = 64 * i + HB * b + 4 * FLUSH_G * gg
                        nc.gpsimd.dma_start(
                            out=out[:, r0:r0 + 4 * FLUSH_G, :],
                            in_=stage[64 * b:64 * b + 64, :],
                        )
    nc.finalize()
    return nc


_NC = None


def _get_nc():
    global _NC
    if _NC is None:
        _NC = build_nc()
    return _NC


def make_in_maps(x, s, weight):
    x = np.asarray(x, dtype=np.float32)
    s = np.asarray(s, dtype=np.float32)
    w = np.asarray(weight, dtype=np.float32)
    wT = w.reshape(COUT, CIN, 9).transpose(1, 2, 0).reshape(CIN, 9 * COUT)
    wT2 = np.ascontiguousarray(np.concatenate([wT, wT], axis=0))  # [128, 576]
    xpad = np.zeros((B, CIN, PH, PW), dtype=ml_dtypes.bfloat16)
    xpad[:, :, 1:H + 1, 1:W + 1] = x
    maps = []
    for c in range(B):
        sT2 = np.ascontiguousarray(
            np.tile(s[c][:, None], (2, 1)).astype(np.float32))  # [128, 1]
        maps.append({"xp": xpad[c], "sT": sT2, "wgtT": wT2})
    return maps


def run(x, s, weight, **kw):
    nc = _get_nc()
    res = run_bass_kernel_spmd(nc, make_in_maps(x, s, weight),
                               core_ids=list(range(B)), **kw)
    out = np.stack([np.asarray(r["out"]) for r in res.results])
    return out, res


def kernel(x, s, weight):
    out, _ = run(x, s, weight)
    return out.astype(np.float32)


if __name__ == "__main__":
    rng = np.random.default_rng(0)
    xv = rng.standard_normal((B, CIN, H, W), dtype=np.float32)
    sv = rng.standard_normal((B, CIN), dtype=np.float32)
    wv = (rng.standard_normal((COUT, CIN, KK, KK), dtype=np.float32)
          * np.float32(np.sqrt(2.0 / (CIN * KK * KK))))
    o = kernel(xv, sv, wv)
    print("ran ok", o.shape, o.dtype, float(np.abs(o).max()))


# revision 20
# speedup vs baseline: 1.0937x; 1.0077x over previous
"""Trainium2 Bass kernel for nn_Conv2DMod (StyleGAN2-style modulated 3x3 conv).

Problem: x[8,64,256,256], s[8,64], weight[64,64,3,3] (f32)
  w = weight * (s+1) per sample; demod by rsqrt(sum w^2 over (Cin,K,K));
  out[b] = conv2d(x[b], w_b, pad=1).

Sharding: data-parallel over batch. 8 samples -> 8 NeuronCores, one each.

Per-core algorithm (V5):
  - host pre-pads x to [64,258,258] bf16 (zero halo rows+cols), pre-transposes
    weight to lhsT layout [i, p, o] replicated to 128 partitions, s as column.
  - all 4 x row-slab loads are issued first on the sync (HWDGE) queue
    (xpool bufs=4, so none waits on buffer recycling); weight/s prep inputs go
    on the scalar queue so the two never serialize.
  - conv runs with MODULATED but UNdemodulated weights (w2 = wT*(s+1), bf16),
    ready as soon as the weight DMA + one DVE pass finish. The demod scale
    d[o] = rsqrt(sum w^2) is computed concurrently as a [128,1] column
    (duplicated halves, via two K=1 transpose matmuls) and applied during
    evacuation as a per-partition scale.
  - conv as shift-matmul over 9 kernel positions, 4 independent 64x64 PE cells
    (row tiles = block0/block1 x data, col tiles = even/odd kernel positions,
    crossed psum banks). Groups of 2 chunks (2x2 output rows, N=512 each)
    share each LDWEIGHTS; position 8 alternates col groups per chunk so every
    cell runs exactly 9 matmuls per group.
  - psum group tiles span 2 banks; 2 bufs x (E,O) = all 8 banks.
  - evacuation per group, engines balanced under the PE's ~2.2us/group:
      DVE: stageB = psE * d2   (frees psE early, independent of ACT)
      ACT: stage[0:64]  = psO[64:128] * d2 (cross-base copy w/ scale)
      ACT: stage[64:128] = psO[0:64]  * d2 (frees psO)
      DVE: stage += stageB     (aligned full-width bf16 add, 2x rate)
    Output DMA'd as bf16 (host upcasts); stores on SWDGE (gpsimd queue).
"""

import ml_dtypes
import numpy as np

import concourse.bacc as bacc
import concourse.mybir as mybir
import concourse.tile as tile
from concourse.bass import ts
from concourse.bass_utils import run_bass_kernel_spmd

F32 = mybir.dt.float32
BF16 = mybir.dt.bfloat16

B, CIN, COUT, KK, H, W = 8, 64, 64, 3, 256, 256
EPS = 1e-8
PW = W + 2          # padded row width
PH = H + 2          # padded height
HB = 32             # output rows per block
NBI = H // (2 * HB)  # pair-iterations (4)
NGRP = HB // 4      # 2-chunk groups per iteration (8)
FLUSH_G = 2         # groups per stage flush (4 chunks = 8 rows per block)

EVEN = [0, 2, 4, 6]
ODD = [1, 3, 5, 7]


def emit_x_load(nc, xt, xp, i):
    lo0 = 64 * i          # block0 padded rows [lo0, lo0+34)
    lo1 = 64 * i + HB     # block1 padded rows [lo1, lo1+34)
    if i == 0:
        # split so the first groups' rows land fast
        nc.sync.dma_start(out=xt[0:64, 0:12, :], in_=xp[:, 0:12, :])
        nc.sync.dma_start(out=xt[64:128, 0:12, :], in_=xp[:, lo1:lo1 + 12, :])
        nc.sync.dma_start(out=xt[0:64, 12:HB + 2, :], in_=xp[:, 12:HB + 2, :])
        nc.sync.dma_start(out=xt[64:128, 12:HB + 2, :],
                          in_=xp[:, lo1 + 12:lo1 + HB + 2, :])
    else:
        nc.sync.dma_start(out=xt[0:64, :, :], in_=xp[:, lo0:lo0 + HB + 2, :])
        nc.sync.dma_start(out=xt[64:128, :, :], in_=xp[:, lo1:lo1 + HB + 2, :])


def build_nc():
    nc = bacc.Bacc("TRN2")
    xp = nc.dram_tensor("xp", [CIN, PH, PW], BF16, kind="ExternalInput")
    sT = nc.dram_tensor("sT", [128, 1], F32, kind="ExternalInput")
    wgtT = nc.dram_tensor("wgtT", [128, 9 * 64], F32, kind="ExternalInput")
    out = nc.dram_tensor("out", [COUT, H, W], BF16, kind="ExternalOutput")

    with (
        tile.TileContext(nc) as tc,
        tc.tile_pool(name="const", bufs=1) as constp,
        tc.tile_pool(name="xpool", bufs=NBI) as xpool,
        tc.tile_pool(name="prep", bufs=1) as prepp,
    ):
        w2 = constp.tile([128, 9, 64], BF16)   # [i, p, o] modulated lhsT
        d2 = constp.tile([128, 1], F32)        # demod scale column (dup halves)

        # weight/s inputs first (small), then all x slab loads, in program
        # order on the sync HWDGE queue which streams them from t=0
        wT = prepp.tile([128, 9, 64], F32)    # [i, p, o]
        nc.sync.dma_start(out=wT[:, :, :], in_=wgtT[:, :])
        sp1 = prepp.tile([128, 1], F32)
        nc.sync.dma_start(out=sp1[:, :], in_=sT[:, :])
        xts = []
        for i in range(NBI):
            xts.append(xpool.tile([128, HB + 2, PW], BF16, name=f"xt{i}",
                                  tag="xt"))
            emit_x_load(nc, xts[i], xp, i)

        # ---- weight prep (f32 math, bf16 result), no transposes ----
        # w2 (bf16) is produced LAST so the conv LDWEIGHTS only become ready
        # after the prep matmuls are already in the tensor queue; otherwise
        # Tile schedules the conv stream first and the first evacuations
        # stall on d2.
        with (
            tc.tile_pool(name="prep_ps", bufs=1, space="PSUM") as prep_ps,
        ):
            nc.vector.tensor_scalar_add(sp1[:, :], sp1[:, :], 1.0)
            wmodF = prepp.tile([128, 9, 64], F32)
            nc.vector.tensor_scalar_mul(wmodF[:, :, :], wT[:, :, :],
                                        sp1[:, :])
            # demod norm: d2[o] = rsqrt(sum_i sum_p wmod^2 + eps) as a column
            sq = prepp.tile([64, 9, 64], F32)
            nc.vector.tensor_mul(sq[:, :, :], wmodF[0:64, :, :],
                                 wmodF[0:64, :, :])
            ones = prepp.tile([64, 1], F32)
            nc.vector.memset(ones[:, :], 1.0)
            psA = prep_ps.tile([64, 512], F32)
            psB = prep_ps.tile([64, 64], F32)
            nc.tensor.matmul(psA[0:1, :], ones[:, 0:1], sq[:, 0:8, :],
                             start=True, stop=True)
            nc.tensor.matmul(psB[0:1, :], ones[:, 0:1], sq[:, 8, :],
                             start=True, stop=True)
            acc = prepp.tile([1, 64], F32)
            nc.vector.tensor_copy(acc[0:1, :], psA[0:1, 0:64])
            for k in range(1, 8):
                nc.vector.tensor_add(acc[0:1, :], acc[0:1, :],
                                     psA[0:1, ts(k, 64)])
            nc.vector.tensor_add(acc[0:1, :], acc[0:1, :], psB[0:1, :])
            epst = prepp.tile([1, 1], F32)
            nc.vector.memset(epst[:, :], EPS)
            dtmp = prepp.tile([1, 64], F32)
            nc.scalar.activation(dtmp[0:1, :], acc[0:1, :],
                                 mybir.ActivationFunctionType.Sqrt,
                                 bias=epst[0:1, 0:1])
            dinv = prepp.tile([1, 64], F32)
            nc.vector.reciprocal(dinv[0:1, :], dtmp[0:1, :])
            # row -> column (both halves) via K=1 transpose matmuls
            ones1 = prepp.tile([1, 1], F32)
            nc.vector.memset(ones1[:, :], 1.0)
            psD = prep_ps.tile([128, 1], F32)
            nc.tensor.matmul(psD[0:64, 0:1], dinv[0:1, :], ones1[0:1, :],
                             start=True, stop=True)
            nc.tensor.matmul(psD[64:128, 0:1], dinv[0:1, :], ones1[0:1, :],
                             start=True, stop=True, tile_position=(0, 64))
            nc.vector.tensor_copy(d2[:, :], psD[:, :])
            # conv weights (bf16), demod applied at evacuation instead
            nc.vector.tensor_copy(w2[:, :, :], wmodF[:, :, :])

        # ---- main conv loop ----
        with (
            tc.tile_pool(name="stpool", bufs=2) as stpool,
            tc.tile_pool(name="sbpool", bufs=2) as sbpool,
            tc.tile_pool(name="pspool", bufs=2, space="PSUM") as pspool,
        ):
            for i in range(NBI):
                xt = xts[i]
                for gg in range(NGRP // FLUSH_G):
                    stage = stpool.tile([128, FLUSH_G * 1024], BF16,
                                        name=f"stage{i}_{gg}", tag="stage")
                    for gj in range(FLUSH_G):
                        g = gg * FLUSH_G + gj
                        psE = pspool.tile([128, 1024], F32,
                                          name=f"psE{i}_{g}", tag="psE")
                        psO = pspool.tile([128, 1024], F32,
                                          name=f"psO{i}_{g}", tag="psO")
                        # cells: (b, col h0)=even pos, (b, col h64)=odd pos
                        #  b0 even->psE[0:64], b1 even->psO[0:64]
                        #  b0 odd ->psO[64:128], b1 odd->psE[64:128]
                        for k in range(5):
                            for par in range(2):   # 0=col h0, 1=col h64
                                if k == 4:
                                    p = 8
                                    jjs = [par]    # p=8: jj0 on h0, jj1 on h64
                                else:
                                    p = (EVEN, ODD)[par][k]
                                    jjs = [0, 1]
                                dy, dx = divmod(p, 3)
                                for b in range(2):
                                    if par == 0:
                                        ps = (psE, psO)[b]
                                        pr = slice(0, 64)
                                        tp = (64 * b, 0)
                                    else:
                                        ps = (psO, psE)[b]
                                        pr = slice(64, 128)
                                        tp = (64 * b, 64)
                                    wap = w2[64 * b:64 * b + 64, p, :]
                                    nc.tensor.ldweights(wap, tile_position=tp)
                                    for jj in jjs:
                                        c = 2 * g + jj
                                        # evens: jj0 k0..4 (5), jj1 k0..3 (4)
                                        # odds:  jj0 k0..3 (4), jj1 k0..4 (5)
                                        st = (k == 0)
                                        if par == 0:
                                            sp = (k == 4) if jj == 0 else (k == 3)
                                        else:
                                            sp = (k == 3) if jj == 0 else (k == 4)
                                        nc.tensor.matmul(
                                            ps[pr, ts(jj, 512)], wap,
                                            xt[64 * b:64 * b + 64,
                                               2 * c + dy:2 * c + dy + 2,
                                               dx:dx + W],
                                            start=st, stop=sp,
                                            tile_position=tp,
                                        )
                        # evacuate group: stage rows [4*gj .. 4*gj+4)
                        dst = stage[:, ts(gj, 1024)]
                        stageB = sbpool.tile([128, 1024], BF16,
                                             name=f"stgB{i}_{g}", tag="stgB")
                        nc.vector.tensor_scalar_mul(stageB[:, :], psE[:, :],
                                                    d2[:, :])
                        nc.scalar.activation(dst[0:64, :], psO[64:128, :],
                                             mybir.ActivationFunctionType.Copy,
                                             scale=d2[64:128, :])
                        nc.scalar.activation(dst[64:128, :], psO[0:64, :],
                                             mybir.ActivationFunctionType.Copy,
                                             scale=d2[0:64, :])
                        nc.vector.tensor_add(dst, dst, stageB[:, :])
                    # flush: one DMA per block, 8 rows x 256 each
                    for b in range(2):
                        r0 = 64 * i + HB * b + 4 * FLUSH_G * gg
                        nc.gpsimd.dma_start(
                            out=out[:, r0:r0 + 4 * FLUSH_G, :],
                            in_=stage[64 * b:64 * b + 64, :],
                        )
    nc.finalize()
    return nc


_NC = None


def _get_nc():
    global _NC
    if _NC is None:
        _NC = build_nc()
    return _NC


def make_in_maps(x, s, weight):
    x = np.asarray(x, dtype=np.float32)
    s = np.asarray(s, dtype=np.float32)
    w = np.asarray(weight, dtype=np.float32)
    wT = w.reshape(COUT, CIN, 9).transpose(1, 2, 0).reshape(CIN, 9 * COUT)
    wT2 = np.ascontiguousarray(np.concatenate([wT, wT], axis=0))  # [128, 576]
    xpad = np.zeros((B, CIN, PH, PW), dtype=ml_dtypes.bfloat16)
    xpad[:, :, 1:H + 1, 1:W + 1] = x
    maps = []
    for c in range(B):
        sT2 = np.ascontiguousarray(
            np.tile(s[c][:, None], (2, 1)).astype(np.float32))  # [128, 1]
        maps.append({"xp": xpad[c], "sT": sT2, "wgtT": wT2})
    return maps


def run(x, s, weight, **kw):
    nc = _get_nc()
    res = run_bass_kernel_spmd(nc, make_in_maps(x, s, weight),
                               core_ids=list(range(B)), **kw)
    out = np.stack([np.asarray(r["out"]) for r in res.results])
    return out, res


def kernel(x, s, weight):
    out, _ = run(x, s, weight)
    return out.astype(np.float32)


if __name__ == "__main__":
    rng = np.random.default_rng(0)
    xv = rng.standard_normal((B, CIN, H, W), dtype=np.float32)
    sv = rng.standard_normal((B, CIN), dtype=np.float32)
    wv = (rng.standard_normal((COUT, CIN, KK, KK), dtype=np.float32)
          * np.float32(np.sqrt(2.0 / (CIN * KK * KK))))
    o = kernel(xv, sv, wv)
    print("ran ok", o.shape, o.dtype, float(np.abs(o).max()))
